# revision 4
# baseline (speedup 1.0000x reference)
"""Trainium2 Bass kernel for DLRA conv layer (3x3 low-rank conv + bias + relu).

Computes: relu(extract_patches_3x3(x) @ U @ S @ V + bias) for the step-selected
factor set. Sharded over H across 8 NeuronCores (28 rows each, 1-px halo
resolved on host). The small factor product (S @ V) is folded on host.

Device dataflow per core (per image, per 4-row group):
  stage 1: 9 shifted matmuls (K=64 channels each) accumulate the 3x3 conv
           576->100 into PSUM (M=rank, N=448 pixels = 2 rows).
  stage 2: z1 (rank x pixels, SBUF) becomes the stationary operand so the
           100->256 matmul emits (pixels x filters) directly -> PSUM.
  epilogue: DVE adds the per-pixel bias, ACT applies relu, DMA stores
           contiguous (pixels x 256) blocks.
"""

import numpy as np
from contextlib import ExitStack

import concourse.bacc as bacc
import concourse.tile as tile
import concourse.mybir as mybir
from concourse.bass_utils import run_bass_kernel_spmd

B, H, W, C = 8, 224, 224, 64
KH = KW = 3
RANK = 100
FILTERS = 256
IN_DIM = KH * KW * C  # 576

NCORES = 8
HS = H // NCORES          # 28 output rows per core
HSH = HS + 2              # input rows incl halo
WP = W + 2                # padded width
NPIX = HS * W             # 6272 pixels per image strip
PCHUNK = 128              # pixels per stage-2 matmul (partition dim)
NCHUNK = NPIX // PCHUNK   # 49
GROUP_ROWS = 4            # output rows per stage-1/2 group
NG = HS // GROUP_ROWS     # 7 groups
PAIR_PIX = 2 * W          # 448 pixels per stage-1 matmul (2 rows)
GPIX = GROUP_ROWS * W     # 896 pixels per group
GCHUNK = GPIX // PCHUNK   # 7 chunks per group

F32 = mybir.dt.float32
MM_DT = mybir.dt.float32r  # matmul operand dtype (full-rate fp32 path)

_CACHE = {}


def _build_nc(mm_dt=MM_DT):
    nc = bacc.Bacc("TRN2", target_bir_lowering=False, debug=False,
                   num_devices=NCORES)
    xt = nc.dram_tensor("xt", [B, C, HSH * WP], mm_dt,
                        kind="ExternalInput").ap()
    w1 = nc.dram_tensor("w1", [C, 9 * RANK], mm_dt, kind="ExternalInput").ap()
    w2 = nc.dram_tensor("w2", [RANK, FILTERS], mm_dt,
                        kind="ExternalInput").ap()
    bias = nc.dram_tensor("bias", [PCHUNK, NCHUNK * FILTERS], F32,
                          kind="ExternalInput").ap()
    out = nc.dram_tensor("out", [B, NCHUNK, PCHUNK, FILTERS], F32,
                         kind="ExternalOutput").ap()
    relu = mybir.ActivationFunctionType.Relu

    with tile.TileContext(nc) as tc, ExitStack() as ctx:
        const = ctx.enter_context(tc.tile_pool(name="const", bufs=1))
        xpool = ctx.enter_context(tc.tile_pool(name="xpool", bufs=2))
        z1pool = ctx.enter_context(tc.tile_pool(name="z1pool", bufs=3))
        ps1pool = ctx.enter_context(
            tc.tile_pool(name="ps1", bufs=4, space="PSUM"))
        ps2pool = ctx.enter_context(
            tc.tile_pool(name="ps2", bufs=4, space="PSUM"))
        opool = ctx.enter_context(tc.tile_pool(name="opool", bufs=3))
        tpool = ctx.enter_context(tc.tile_pool(name="tpool", bufs=4))

        w1_t = const.tile([C, 9 * RANK], mm_dt, name="w1_t")
        nc.sync.dma_start(w1_t[:], w1[:])
        w2_t = const.tile([RANK, FILTERS], mm_dt, name="w2_t")
        nc.sync.dma_start(w2_t[:], w2[:])
        bias_t = const.tile([PCHUNK, NCHUNK * FILTERS], F32, name="bias_t")
        nc.sync.dma_start(bias_t[:], bias[:])

        w2_mm = w2_t[:]

        def stage1(xtv, g):
            """Conv 576->100 for 4 output rows; returns z1 tile."""
            z1 = z1pool.tile([RANK, GPIX], mm_dt, name="z1", tag="z1")
            for hp in range(2):  # row pairs within the group
                r0 = g * GROUP_ROWS + 2 * hp
                ps1 = ps1pool.tile([RANK, PAIR_PIX], F32, name="ps1",
                                   tag="ps1")
                for j in range(9):
                    di, dj = divmod(j, 3)
                    rhs = xtv[:, r0 + di:r0 + di + 2, dj:dj + W]
                    nc.tensor.matmul(
                        ps1[:],
                        lhsT=w1_t[:, j * RANK:(j + 1) * RANK],
                        rhs=rhs,
                        start=(j == 0),
                        stop=(j == 8),
                    )
                nc.vector.tensor_copy(
                    z1[:, hp * PAIR_PIX:(hp + 1) * PAIR_PIX], ps1[:])
            return z1

        def stage2(img, g, z1):
            """100->256 matmul + bias + relu + store for one group."""
            og = opool.tile([PCHUNK, GCHUNK * FILTERS], F32, name="og",
                            tag="og")
            for kc in range(GCHUNK):
                ps2 = ps2pool.tile([PCHUNK, FILTERS], F32, name="ps2",
                                   tag="ps2")
                nc.tensor.matmul(
                    ps2[:],
                    lhsT=z1[:, kc * PCHUNK:(kc + 1) * PCHUNK],
                    rhs=w2_mm,
                    start=True,
                    stop=True,
                )
                n = g * GCHUNK + kc
                tmp = tpool.tile([PCHUNK, FILTERS], F32, name="tmp", tag="tmp")
                nc.vector.tensor_add(
                    tmp[:], ps2[:], bias_t[:, n * FILTERS:(n + 1) * FILTERS])
                nc.scalar.activation(
                    og[:, kc * FILTERS:(kc + 1) * FILTERS], tmp[:], relu)
            dst = out[img, g * GCHUNK:(g + 1) * GCHUNK, :, :].transpose(
                [1, 0, 2])
            nc.sync.dma_start(dst, og[:].rearrange(
                "p (n f) -> p n f", f=FILTERS))

        # Software-pipelined so PE never waits on the z1 PSUM->SBUF copy:
        # stage2(g) is emitted after stage1(g+1).
        pending = None  # (img, g, z1)
        for img in range(B):
            xt_t = xpool.tile([C, HSH * WP], mm_dt, name="xt_t", tag="xt")
            nc.sync.dma_start(xt_t[:], xt[img])
            xtv = xt_t[:].rearrange("c (r w) -> c r w", w=WP)
            for g in range(NG):
                z1 = stage1(xtv, g)
                if pending is not None:
                    stage2(*pending)
                pending = (img, g, z1)
        stage2(*pending)

    nc.compile()
    return nc


def _get_nc():
    if "nc" not in _CACHE:
        _CACHE["nc"] = _build_nc()
    return _CACHE["nc"]


def _prep_inputs(x, k, l_t, s, aux_U, aux_Unp1, aux_Vt, aux_Vtnp1, b, aux_b,
                 step):
    step = int(np.asarray(step))
    x = np.ascontiguousarray(np.asarray(x, dtype=np.float32))
    if step == 0:
        U, W2, bias = np.asarray(k), np.asarray(aux_Vt), np.asarray(aux_b)
    elif step == 1:
        U, W2, bias = np.asarray(aux_U), np.asarray(l_t), np.asarray(aux_b)
    else:
        U = np.asarray(aux_Unp1)
        W2 = (np.asarray(s, np.float64) @ np.asarray(aux_Vtnp1, np.float64))
        bias = np.asarray(b)
    U = U.astype(np.float32)
    W2 = np.ascontiguousarray(W2.astype(np.float32))
    bias = bias.astype(np.float32)

    # channel-major, zero-padded H and W
    xpad = np.zeros((B, H + 2, W + 2, C), np.float32)
    xpad[:, 1:-1, 1:-1, :] = x
    xpad_t = np.ascontiguousarray(xpad.transpose(0, 3, 1, 2))  # (B,C,226,226)

    w1 = np.ascontiguousarray(
        U.reshape(9, C, RANK).transpose(1, 0, 2).reshape(C, 9 * RANK))

    in_maps = []
    for i in range(NCORES):
        xt_i = np.ascontiguousarray(
            xpad_t[:, :, HS * i:HS * i + HSH, :]).reshape(B, C, HSH * WP)
        b_i = np.ascontiguousarray(
            bias[HS * i:HS * (i + 1)].reshape(NCHUNK, PCHUNK, FILTERS)
            .transpose(1, 0, 2)).reshape(PCHUNK, NCHUNK * FILTERS)
        in_maps.append({"xt": xt_i, "w1": w1, "w2": W2, "bias": b_i})
    return in_maps


def _assemble(results):
    strips = [
        results[i]["out"].reshape(B, HS, W, FILTERS) for i in range(NCORES)
    ]
    return np.ascontiguousarray(np.concatenate(strips, axis=1))


def run(trace=False, **inputs):
    in_maps = _prep_inputs(**inputs)
    nc = _get_nc()
    res = run_bass_kernel_spmd(nc, in_maps, list(range(NCORES)), trace=trace)
    return _assemble(res.results), res


def kernel(**inputs):
    out, _ = run(trace=False, **inputs)
    return out


# revision 5
# speedup vs baseline: 1.2286x; 1.2286x over previous
"""Trainium2 Bass kernel for DLRA conv layer (3x3 low-rank conv + bias + relu).

Computes: relu(extract_patches_3x3(x) @ U @ S @ V + bias) for the step-selected
factor set. Sharded over H across 8 NeuronCores (28 rows each, 1-px halo
resolved on host). The small factor product (S @ V) is folded on host.

Device dataflow per core (per image, per 4-row group):
  stage 1: 9 shifted matmuls (K=64 channels each) accumulate the 3x3 conv
           576->100 into PSUM (M=rank, N=448 pixels = 2 rows).
  stage 2: z1 (rank x pixels, SBUF) becomes the stationary operand so the
           100->256 matmul emits (pixels x filters) directly -> PSUM.
  epilogue: DVE adds the per-pixel bias, ACT applies relu, DMA stores
           contiguous (pixels x 256) blocks.
"""

import numpy as np
from contextlib import ExitStack

import concourse.bacc as bacc
import concourse.tile as tile
import concourse.mybir as mybir
from concourse.bass_utils import run_bass_kernel_spmd

B, H, W, C = 8, 224, 224, 64
KH = KW = 3
RANK = 100
FILTERS = 256
IN_DIM = KH * KW * C  # 576

NCORES = 8
HS = H // NCORES          # 28 output rows per core
HSH = HS + 2              # input rows incl halo
WP = W + 2                # padded width
NPIX = HS * W             # 6272 pixels per image strip
PCHUNK = 128              # pixels per stage-2 matmul (partition dim)
NCHUNK = NPIX // PCHUNK   # 49
GROUP_ROWS = 4            # output rows per stage-1/2 group
NG = HS // GROUP_ROWS     # 7 groups
PAIR_PIX = 2 * W          # 448 pixels per stage-1 matmul (2 rows)
GPIX = GROUP_ROWS * W     # 896 pixels per group
GCHUNK = GPIX // PCHUNK   # 7 chunks per group

F32 = mybir.dt.float32
# fp16 matmul operands: full-rate PE (1 cycle/row @2.4GHz vs fp32r's half-rate
# fp32_mode=HIGH pass), 11-bit mantissa (~4x tighter than bf16), fp32 PSUM
# accumulate. Host casts x/U/W2 to fp16; bias add + output stay fp32.
MM_DT = mybir.dt.float16
MM_NP = {mybir.dt.float16: np.float16,
         mybir.dt.bfloat16: None,  # needs ml_dtypes
         mybir.dt.float32r: np.float32,
         mybir.dt.float32: np.float32}

_CACHE = {}


def _build_nc(mm_dt=MM_DT):
    nc = bacc.Bacc("TRN2", target_bir_lowering=False, debug=False,
                   num_devices=NCORES)
    xt = nc.dram_tensor("xt", [B, C, HSH * WP], mm_dt,
                        kind="ExternalInput").ap()
    w1 = nc.dram_tensor("w1", [C, 9 * RANK], mm_dt, kind="ExternalInput").ap()
    w2 = nc.dram_tensor("w2", [RANK, FILTERS], mm_dt,
                        kind="ExternalInput").ap()
    bias = nc.dram_tensor("bias", [PCHUNK, NCHUNK * FILTERS], F32,
                          kind="ExternalInput").ap()
    out = nc.dram_tensor("out", [B, NCHUNK, PCHUNK, FILTERS], F32,
                         kind="ExternalOutput").ap()
    relu = mybir.ActivationFunctionType.Relu

    with tile.TileContext(nc) as tc, ExitStack() as ctx:
        const = ctx.enter_context(tc.tile_pool(name="const", bufs=1))
        xpool = ctx.enter_context(tc.tile_pool(name="xpool", bufs=2))
        z1pool = ctx.enter_context(tc.tile_pool(name="z1pool", bufs=3))
        ps1pool = ctx.enter_context(
            tc.tile_pool(name="ps1", bufs=4, space="PSUM"))
        ps2pool = ctx.enter_context(
            tc.tile_pool(name="ps2", bufs=4, space="PSUM"))
        opool = ctx.enter_context(tc.tile_pool(name="opool", bufs=3))
        tpool = ctx.enter_context(tc.tile_pool(name="tpool", bufs=4))

        w1_t = const.tile([C, 9 * RANK], mm_dt, name="w1_t")
        nc.sync.dma_start(w1_t[:], w1[:])
        w2_t = const.tile([RANK, FILTERS], mm_dt, name="w2_t")
        nc.sync.dma_start(w2_t[:], w2[:])
        bias_t = const.tile([PCHUNK, NCHUNK * FILTERS], F32, name="bias_t")
        nc.sync.dma_start(bias_t[:], bias[:])

        w2_mm = w2_t[:]

        def stage1(xtv, g):
            """Conv 576->100 for 4 output rows; returns z1 tile."""
            z1 = z1pool.tile([RANK, GPIX], mm_dt, name="z1", tag="z1")
            for hp in range(2):  # row pairs within the group
                r0 = g * GROUP_ROWS + 2 * hp
                ps1 = ps1pool.tile([RANK, PAIR_PIX], F32, name="ps1",
                                   tag="ps1")
                for j in range(9):
                    di, dj = divmod(j, 3)
                    rhs = xtv[:, r0 + di:r0 + di + 2, dj:dj + W]
                    nc.tensor.matmul(
                        ps1[:],
                        lhsT=w1_t[:, j * RANK:(j + 1) * RANK],
                        rhs=rhs,
                        start=(j == 0),
                        stop=(j == 8),
                    )
                nc.vector.tensor_copy(
                    z1[:, hp * PAIR_PIX:(hp + 1) * PAIR_PIX], ps1[:])
            return z1

        def stage2(img, g, z1):
            """100->256 matmul + bias + relu + store for one group."""
            og = opool.tile([PCHUNK, GCHUNK * FILTERS], F32, name="og",
                            tag="og")
            for kc in range(GCHUNK):
                ps2 = ps2pool.tile([PCHUNK, FILTERS], F32, name="ps2",
                                   tag="ps2")
                nc.tensor.matmul(
                    ps2[:],
                    lhsT=z1[:, kc * PCHUNK:(kc + 1) * PCHUNK],
                    rhs=w2_mm,
                    start=True,
                    stop=True,
                )
                n = g * GCHUNK + kc
                tmp = tpool.tile([PCHUNK, FILTERS], F32, name="tmp", tag="tmp")
                nc.vector.tensor_add(
                    tmp[:], ps2[:], bias_t[:, n * FILTERS:(n + 1) * FILTERS])
                nc.scalar.activation(
                    og[:, kc * FILTERS:(kc + 1) * FILTERS], tmp[:], relu)
            dst = out[img, g * GCHUNK:(g + 1) * GCHUNK, :, :].transpose(
                [1, 0, 2])
            nc.sync.dma_start(dst, og[:].rearrange(
                "p (n f) -> p n f", f=FILTERS))

        # Software-pipelined so PE never waits on the z1 PSUM->SBUF copy:
        # stage2(g) is emitted after stage1(g+1).
        pending = None  # (img, g, z1)
        for img in range(B):
            xt_t = xpool.tile([C, HSH * WP], mm_dt, name="xt_t", tag="xt")
            nc.sync.dma_start(xt_t[:], xt[img])
            xtv = xt_t[:].rearrange("c (r w) -> c r w", w=WP)
            for g in range(NG):
                z1 = stage1(xtv, g)
                if pending is not None:
                    stage2(*pending)
                pending = (img, g, z1)
        stage2(*pending)

    nc.compile()
    return nc


def _get_nc():
    if "nc" not in _CACHE:
        _CACHE["nc"] = _build_nc()
    return _CACHE["nc"]


def _prep_inputs(x, k, l_t, s, aux_U, aux_Unp1, aux_Vt, aux_Vtnp1, b, aux_b,
                 step):
    step = int(np.asarray(step))
    x = np.ascontiguousarray(np.asarray(x, dtype=np.float32))
    if step == 0:
        U, W2, bias = np.asarray(k), np.asarray(aux_Vt), np.asarray(aux_b)
    elif step == 1:
        U, W2, bias = np.asarray(aux_U), np.asarray(l_t), np.asarray(aux_b)
    else:
        U = np.asarray(aux_Unp1)
        W2 = (np.asarray(s, np.float64) @ np.asarray(aux_Vtnp1, np.float64))
        bias = np.asarray(b)
    U = U.astype(np.float32)
    W2 = np.ascontiguousarray(W2.astype(np.float32))
    bias = bias.astype(np.float32)

    # channel-major, zero-padded H and W
    xpad = np.zeros((B, H + 2, W + 2, C), np.float32)
    xpad[:, 1:-1, 1:-1, :] = x
    xpad_t = np.ascontiguousarray(xpad.transpose(0, 3, 1, 2))  # (B,C,226,226)

    w1 = np.ascontiguousarray(
        U.reshape(9, C, RANK).transpose(1, 0, 2).reshape(C, 9 * RANK))

    mm_np = MM_NP[MM_DT]
    if mm_np is None:
        import ml_dtypes
        mm_np = ml_dtypes.bfloat16
    w1 = w1.astype(mm_np)
    W2 = W2.astype(mm_np)
    in_maps = []
    for i in range(NCORES):
        xt_i = np.ascontiguousarray(
            xpad_t[:, :, HS * i:HS * i + HSH, :]).reshape(
                B, C, HSH * WP).astype(mm_np)
        b_i = np.ascontiguousarray(
            bias[HS * i:HS * (i + 1)].reshape(NCHUNK, PCHUNK, FILTERS)
            .transpose(1, 0, 2)).reshape(PCHUNK, NCHUNK * FILTERS)
        in_maps.append({"xt": xt_i, "w1": w1, "w2": W2, "bias": b_i})
    return in_maps


def _assemble(results):
    strips = [
        results[i]["out"].reshape(B, HS, W, FILTERS) for i in range(NCORES)
    ]
    return np.ascontiguousarray(np.concatenate(strips, axis=1))


def run(trace=False, **inputs):
    in_maps = _prep_inputs(**inputs)
    nc = _get_nc()
    res = run_bass_kernel_spmd(nc, in_maps, list(range(NCORES)), trace=trace)
    return _assemble(res.results), res


def kernel(**inputs):
    out, _ = run(trace=False, **inputs)
    return out


# revision 6
# speedup vs baseline: 1.9864x; 1.6169x over previous
"""Trainium2 Bass kernel for DLRA conv layer (3x3 low-rank conv + bias + relu).

Computes: relu(extract_patches_3x3(x) @ U @ S @ V + bias) for the step-selected
factor set. Sharded over H across 8 NeuronCores (28 rows each, 1-px halo
resolved on host). The small factor product (S @ V) is folded on host.

Device dataflow per core (per image, per 4-row group):
  stage 1: the 9 conv shifts are packed into 5 K=128 matmuls (K=64 runs at
           half PE rate on trn2). Two 128-partition copies of the image hold
           [x; x shifted 1 col] (bufA) and [x; x shifted 1 row] (bufB), so a
           single matmul contracts two shifts at once:
             p=0..2: shifts (di,0)+(di,1) via bufA, di=0..2
             p=3:    shifts (0,2)+(1,2)  via bufB
             p=4:    shift  (2,2) via bufA top, bottom weights zeroed
           Accumulated into PSUM (M=rank100, N=448 px = 2 rows).
  stage 2: z1 (rank x pixels, fp16 SBUF) is the stationary operand so the
           100->256 matmul emits (pixels x filters) directly -> PSUM, two
           chunks batched per 2KB PSUM bank.
  epilogue: DVE adds the per-pixel bias (fp32), ACT applies relu, DMA stores
           contiguous (pixels x 256) blocks.

fp16 matmul operands: full PE rate, fp32 PSUM accumulate, ~4e-4 rel err.
"""

import numpy as np
from contextlib import ExitStack

import concourse.bacc as bacc
import concourse.tile as tile
import concourse.mybir as mybir
from concourse.bass_utils import run_bass_kernel_spmd

B, H, W, C = 8, 224, 224, 64
KH = KW = 3
RANK = 100
FILTERS = 256
IN_DIM = KH * KW * C  # 576

NCORES = 8
HS = H // NCORES          # 28 output rows per core
HSH = HS + 2              # input rows incl halo
WP = W + 2                # padded width
XL = HSH * WP             # flat image-strip length per channel (6780)
NPIX = HS * W             # 6272 pixels per image strip
PCHUNK = 128              # pixels per stage-2 matmul (partition dim)
NCHUNK = NPIX // PCHUNK   # 49
GROUP_ROWS = 4            # output rows per stage-1/2 group
NG = HS // GROUP_ROWS     # 7 groups
PAIR_PIX = 2 * W          # 448 pixels per stage-1 matmul (2 rows)
GPIX = GROUP_ROWS * W     # 896 pixels per group
GCHUNK = GPIX // PCHUNK   # 7 chunks per group

# stage-1 weight pairs: (top block, bottom block) by shift j = di*3+dj
W1_PAIRS = [(0, 1), (3, 4), (6, 7), (2, 5), (8, None)]
NP1 = len(W1_PAIRS)

F32 = mybir.dt.float32
MM_DT = mybir.dt.float16
MM_NP = np.float16

_CACHE = {}


def _build_nc():
    nc = bacc.Bacc("TRN2", target_bir_lowering=False, debug=False,
                   num_devices=NCORES)
    xt = nc.dram_tensor("xt", [B, C, XL], MM_DT, kind="ExternalInput").ap()
    w1 = nc.dram_tensor("w1", [2 * C, NP1 * RANK], MM_DT,
                        kind="ExternalInput").ap()
    w2 = nc.dram_tensor("w2", [RANK, FILTERS], MM_DT,
                        kind="ExternalInput").ap()
    bias = nc.dram_tensor("bias", [PCHUNK, NCHUNK * FILTERS], F32,
                          kind="ExternalInput").ap()
    out = nc.dram_tensor("out", [B, NCHUNK, PCHUNK, FILTERS], F32,
                         kind="ExternalOutput").ap()
    relu = mybir.ActivationFunctionType.Relu
    fcopy = mybir.ActivationFunctionType.Copy

    with tile.TileContext(nc) as tc, ExitStack() as ctx:
        const = ctx.enter_context(tc.tile_pool(name="const", bufs=1))
        xpool = ctx.enter_context(tc.tile_pool(name="xpool", bufs=2))
        z1pool = ctx.enter_context(tc.tile_pool(name="z1pool", bufs=3))
        ps1pool = ctx.enter_context(
            tc.tile_pool(name="ps1", bufs=4, space="PSUM"))
        ps2pool = ctx.enter_context(
            tc.tile_pool(name="ps2", bufs=4, space="PSUM"))
        opool = ctx.enter_context(tc.tile_pool(name="opool", bufs=3))
        tpool = ctx.enter_context(tc.tile_pool(name="tpool", bufs=4))

        w1_t = const.tile([2 * C, NP1 * RANK], MM_DT, name="w1_t")
        nc.sync.dma_start(w1_t[:], w1[:])
        w2_t = const.tile([RANK, FILTERS], MM_DT, name="w2_t")
        nc.sync.dma_start(w2_t[:], w2[:])
        bias_t = const.tile([PCHUNK, NCHUNK * FILTERS], F32, name="bias_t")
        nc.sync.dma_start(bias_t[:], bias[:])

        def load_image(img):
            """Load strip + build the two shifted 128-partition buffers."""
            bufa = xpool.tile([2 * C, XL], MM_DT, name="bufa", tag="bufa")
            bufb = xpool.tile([2 * C, XL], MM_DT, name="bufb", tag="bufb")
            nc.sync.dma_start(bufa[0:C, :], xt[img])
            # bottom of A: x shifted left 1 col (flat shift; col 225 is pad)
            nc.sync.dma_start(bufa[C:2 * C, 0:XL - 1], bufa[0:C, 1:XL])
            nc.gpsimd.memset(bufa[C:2 * C, XL - 1:XL], 0.0)
            # B: top = x, bottom = x shifted up 1 row
            nc.sync.dma_start(bufb[0:C, :], bufa[0:C, :])
            nc.sync.dma_start(bufb[C:2 * C, 0:XL - WP], bufa[0:C, WP:XL])
            return bufa, bufb

        def stage1(bufa, bufb, g):
            """Conv 576->100 for 4 output rows; returns z1 tile (fp16)."""
            av = bufa[:].rearrange("c (r w) -> c r w", w=WP)
            bv = bufb[:].rearrange("c (r w) -> c r w", w=WP)
            z1 = z1pool.tile([RANK, GPIX], MM_DT, name="z1", tag="z1")
            for hp in range(2):  # row pairs within the group
                r0 = g * GROUP_ROWS + 2 * hp
                ps1 = ps1pool.tile([RANK, PAIR_PIX], F32, name="ps1",
                                   tag="ps1")
                rhss = [
                    av[:, r0 + 0:r0 + 2, 0:W],
                    av[:, r0 + 1:r0 + 3, 0:W],
                    av[:, r0 + 2:r0 + 4, 0:W],
                    bv[:, r0 + 0:r0 + 2, 2:2 + W],
                    av[:, r0 + 2:r0 + 4, 2:2 + W],
                ]
                for p in range(NP1):
                    nc.tensor.matmul(
                        ps1[:],
                        lhsT=w1_t[:, p * RANK:(p + 1) * RANK],
                        rhs=rhss[p],
                        start=(p == 0),
                        stop=(p == NP1 - 1),
                    )
                # PSUM -> SBUF fp16 cast; alternate engines to balance load
                dstz = z1[:, hp * PAIR_PIX:(hp + 1) * PAIR_PIX]
                if hp == 0:
                    nc.vector.tensor_copy(dstz, ps1[:])
                else:
                    nc.scalar.activation(dstz, ps1[:], fcopy)
            return z1

        def stage2(img, g, z1):
            """100->256 matmul + bias + relu + store for one group."""
            og = opool.tile([PCHUNK, GCHUNK * FILTERS], F32, name="og",
                            tag="og")
            kc = 0
            while kc < GCHUNK:
                nb = min(2, GCHUNK - kc)  # chunks batched into one PSUM bank
                ps2 = ps2pool.tile([PCHUNK, 2 * FILTERS], F32, name="ps2",
                                   tag="ps2")
                for i in range(nb):
                    nc.tensor.matmul(
                        ps2[:, i * FILTERS:(i + 1) * FILTERS],
                        lhsT=z1[:, (kc + i) * PCHUNK:(kc + i + 1) * PCHUNK],
                        rhs=w2_t[:],
                        start=True,
                        stop=True,
                        skip_group_check=(i > 0),
                    )
                n = g * GCHUNK + kc
                fs = nb * FILTERS
                tmp = tpool.tile([PCHUNK, 2 * FILTERS], F32, name="tmp",
                                 tag="tmp")
                nc.vector.tensor_add(
                    tmp[:, 0:fs], ps2[:, 0:fs],
                    bias_t[:, n * FILTERS:n * FILTERS + fs])
                nc.scalar.activation(
                    og[:, kc * FILTERS:kc * FILTERS + fs], tmp[:, 0:fs], relu)
                kc += nb
            dst = out[img, g * GCHUNK:(g + 1) * GCHUNK, :, :].transpose(
                [1, 0, 2])
            nc.sync.dma_start(dst, og[:].rearrange(
                "p (n f) -> p n f", f=FILTERS))

        # Software-pipelined so PE never waits on the z1 PSUM->SBUF copy:
        # stage2(g) is emitted after stage1(g+1).
        pending = None  # (img, g, z1)
        for img in range(B):
            bufa, bufb = load_image(img)
            for g in range(NG):
                z1 = stage1(bufa, bufb, g)
                if pending is not None:
                    stage2(*pending)
                pending = (img, g, z1)
        stage2(*pending)

    nc.compile()
    return nc


def _get_nc():
    if "nc" not in _CACHE:
        _CACHE["nc"] = _build_nc()
    return _CACHE["nc"]


def _prep_inputs(x, k, l_t, s, aux_U, aux_Unp1, aux_Vt, aux_Vtnp1, b, aux_b,
                 step):
    step = int(np.asarray(step))
    x = np.ascontiguousarray(np.asarray(x, dtype=np.float32))
    if step == 0:
        U, W2, bias = np.asarray(k), np.asarray(aux_Vt), np.asarray(aux_b)
    elif step == 1:
        U, W2, bias = np.asarray(aux_U), np.asarray(l_t), np.asarray(aux_b)
    else:
        U = np.asarray(aux_Unp1)
        W2 = (np.asarray(s, np.float64) @ np.asarray(aux_Vtnp1, np.float64))
        bias = np.asarray(b)
    U = U.astype(np.float32)
    W2 = np.ascontiguousarray(W2.astype(MM_NP))
    bias = bias.astype(np.float32)

    # channel-major, zero-padded H and W
    xpad = np.zeros((B, H + 2, W + 2, C), np.float32)
    xpad[:, 1:-1, 1:-1, :] = x
    xpad_t = np.ascontiguousarray(xpad.transpose(0, 3, 1, 2))  # (B,C,226,226)

    # stage-1 stationary: vertical stacks of shift-block pairs (128 x 100)
    blocks = U.reshape(9, C, RANK)
    w1p = np.zeros((NP1, 2 * C, RANK), np.float32)
    for p, (jt, jb) in enumerate(W1_PAIRS):
        w1p[p, 0:C] = blocks[jt]
        if jb is not None:
            w1p[p, C:2 * C] = blocks[jb]
    w1 = np.ascontiguousarray(
        w1p.transpose(1, 0, 2).reshape(2 * C, NP1 * RANK)).astype(MM_NP)

    in_maps = []
    for i in range(NCORES):
        xt_i = np.ascontiguousarray(
            xpad_t[:, :, HS * i:HS * i + HSH, :]).reshape(
                B, C, XL).astype(MM_NP)
        b_i = np.ascontiguousarray(
            bias[HS * i:HS * (i + 1)].reshape(NCHUNK, PCHUNK, FILTERS)
            .transpose(1, 0, 2)).reshape(PCHUNK, NCHUNK * FILTERS)
        in_maps.append({"xt": xt_i, "w1": w1, "w2": W2, "bias": b_i})
    return in_maps


def _assemble(results):
    strips = [
        results[i]["out"].reshape(B, HS, W, FILTERS) for i in range(NCORES)
    ]
    return np.ascontiguousarray(np.concatenate(strips, axis=1))


def run(trace=False, **inputs):
    in_maps = _prep_inputs(**inputs)
    nc = _get_nc()
    res = run_bass_kernel_spmd(nc, in_maps, list(range(NCORES)), trace=trace)
    return _assemble(res.results), res


def kernel(**inputs):
    out, _ = run(trace=False, **inputs)
    return out
